# revision 1
# baseline (speedup 1.0000x reference)
"""Neural-CA step kernel for Trainium2, 8-core batch-parallel SPMD.

Strategy (per core, 4 of 32 batch images):
  Pass 1 (MLP): depthwise 3x3 perception conv folded into layer-1 weights
    (3 K=48 matmuls per 512-pixel tile, accumulating in PSUM), then the
    three 1x1 conv layers as K=128 matmuls. fp32r matmuls (full PE rate,
    ~1e-5 precision). leaky-relu on ScalarE (Lrelu ACT, exact 0.01 slope,
    carries bias) and VectorE (scalar_tensor_tensor max(z, 0.01z), exact).
    dy written to DRAM scratch via 4-way PSUM column packing.
  Pass 2 (update + alive masks): channel-major packed layout [8c+o, px];
    x_new = x + dy*mask with mask replicated by a 0-stride DMA; alpha
    channels written to a 258-pitch guarded pad (DRAM) so the 3x3 max-pool
    needs no edge fixups; alive = (|pool a0|+|pool a1| > 0.01) for pre and
    post states; out = x_new * pre * post.
"""

import numpy as np
import bass_rust

from concourse import bacc, tile, mybir
from concourse.bass_utils import run_bass_kernel_spmd

F32 = mybir.dt.float32
F32R = mybir.dt.float32r
BF16 = mybir.dt.bfloat16
AF = mybir.ActivationFunctionType
OP = mybir.AluOpType

B, CH, H, W = 32, 16, 256, 256
HID = 128
L = H * W                      # 65536 pixels per image
NCORES = 8
BPC = B // NCORES              # batches per core
SROWS = 16                     # rows per strip
NSTRIP = H // SROWS            # strips per batch
TPS = SROWS * W // 512         # 512-px tiles per strip (8)
PITCH = W + 2                  # padded row pitch (258)
PADN = PITCH * PITCH + 6       # guarded alpha plane size (66570)
F2 = 2048                      # free size of a pass-2 quarter tile
NQ = 4                         # quarters per batch
NEG = -1.0e30


def _V(dims):
    return bass_rust.VecI64Pair([list(d) for d in dims])


def _surg(ap, dims, extra_offset=0):
    c = ap.copy()
    c.ap = _V(dims)
    if extra_offset:
        c.offset = c.offset + extra_offset
    return c


def _build():
    """Build the SPMD one-step program (biases always applied via ACT)."""
    nc = bacc.Bacc("TRN2", target_bir_lowering=False, debug=False)

    x_d = nc.dram_tensor("x", [BPC * CH * L], F32, kind="ExternalInput").ap()
    mask_d = nc.dram_tensor("mask", [BPC * L], F32, kind="ExternalInput").ap()
    wtap_d = nc.dram_tensor("wtap", [3, 48, HID], F32, kind="ExternalInput").ap()
    w2t_d = nc.dram_tensor("w2t", [HID, HID], F32, kind="ExternalInput").ap()
    w3t_d = nc.dram_tensor("w3t", [HID, HID], F32, kind="ExternalInput").ap()
    w4t_d = nc.dram_tensor("w4t", [HID, CH], BF16, kind="ExternalInput").ap()
    b1_d = nc.dram_tensor("b1", [HID, 1], F32, kind="ExternalInput").ap()
    b2_d = nc.dram_tensor("b2", [HID, 1], F32, kind="ExternalInput").ap()
    b3_d = nc.dram_tensor("b3", [HID, 1], F32, kind="ExternalInput").ap()
    out_d = nc.dram_tensor("out", [BPC * CH * L], F32, kind="ExternalOutput").ap()

    from contextlib import ExitStack
    with ExitStack() as _es:
        tc = _es.enter_context(tile.TileContext(nc))
        _p = lambda **kw: _es.enter_context(tc.tile_pool(**kw))
        wpool = _p(name="wpool", bufs=1)
        x3p = _p(name="x3p", bufs=3)
        h1p = _p(name="h1p", bufs=2)
        h2p = _p(name="h2p", bufs=2)
        h3p = _p(name="h3p", bufs=2)
        dysbp = _p(name="dysbp", bufs=2)
        ps1p = _p(name="ps1p", bufs=1, space="PSUM")
        ps2p = _p(name="ps2p", bufs=1, space="PSUM")
        ps3p = _p(name="ps3p", bufs=1, space="PSUM")
        dy4p = _p(name="dy4p", bufs=2, space="PSUM")
        p2big = _p(name="p2big", bufs=9)
        alphap = _p(name="alphap", bufs=2)
        mhp = _p(name="mhp", bufs=1)
        mvp = _p(name="mvp", bufs=1)
        s01p = _p(name="s01p", bufs=3)
        dram = _p(name="dram", bufs=1, space="DRAM")
        if True:
            # ---------------- scratch DRAM ----------------
            dy_s = dram.tile([BPC * 32 * 4 * 32 * 512], F32)      # padded dy
            x_apad = dram.tile([BPC * 2 * PADN], F32)
            xn_apad = dram.tile([BPC * 2 * PADN], F32)
            alive_lin = dram.tile([BPC * L], F32)

            # ---------------- constants / weights ----------------
            wtap_sb = wpool.tile([48, 3 * HID], F32R)
            for ky in range(3):
                nc.sync.dma_start(wtap_sb[:, ky * HID:(ky + 1) * HID],
                                  wtap_d[ky].bitcast(F32R))
            w2t_sb = wpool.tile([HID, HID], F32R)
            nc.sync.dma_start(w2t_sb[:], w2t_d.bitcast(F32R))
            w3t_sb = wpool.tile([HID, HID], F32R)
            nc.sync.dma_start(w3t_sb[:], w3t_d.bitcast(F32R))
            w4t_sb = wpool.tile([HID, CH], BF16)
            nc.sync.dma_start(w4t_sb[:], w4t_d[:])
            b1_sb = wpool.tile([HID, 1], F32)
            nc.sync.dma_start(b1_sb[:], b1_d[:])
            b2_sb = wpool.tile([HID, 1], F32)
            nc.sync.dma_start(b2_sb[:], b2_d[:])
            b3_sb = wpool.tile([HID, 1], F32)
            nc.sync.dma_start(b3_sb[:], b3_d[:])
            zneg_sb = wpool.tile([1, 1024], F32)
            nc.vector.memset(zneg_sb[:, 0:512], 0.0)
            nc.vector.memset(zneg_sb[:, 512:1024], NEG)
            zn_dram = dram.tile([1024], F32)
            nc.sync.dma_start(_surg(zn_dram[:], [[1, 1024]]),
                              _surg(zneg_sb[:], [[1024, 1], [1, 1024]]))

            def zeros_in(counts):
                # constant-source in-AP (DRAM zeros) matching `counts`
                dims = [[0, c] for c in counts]
                dims[-1] = [1, counts[-1]]
                return _surg(zn_dram[:], dims, 0)

            def neg_in(counts):
                dims = [[0, c] for c in counts]
                dims[-1] = [1, counts[-1]]
                return _surg(zn_dram[:], dims, 512)

            # ---------------- guarded alpha pads: guard fill ----------------
            for b in range(BPC):
                for pad, src_is_x in ((x_apad, True), (xn_apad, False)):
                    for c in range(2):
                        base = (b * 2 + c) * PADN
                        # top pad row + leading guard [0, 260)
                        nc.sync.dma_start(
                            _surg(pad[:], [[1, 260]], base), neg_in([260]))
                        # bottom pad row + tail [PADN-264, PADN)
                        nc.sync.dma_start(
                            _surg(pad[:], [[1, 264]], base + PADN - 264),
                            neg_in([264]))
                        # row-guard pairs
                        nc.sync.dma_start(
                            _surg(pad[:], [[PITCH, 256], [1, 2]],
                                  base + 1 + PITCH + 257),
                            neg_in([256, 2]))
                # x_apad interior: DRAM->DRAM from x alpha channels
                nc.sync.dma_start(
                    _surg(x_apad[:], [[PADN, 2], [PITCH, 256], [1, 256]],
                          b * 2 * PADN + 1 + PITCH + 1),
                    _surg(x_d, [[L, 2], [256, 256], [1, 256]], b * CH * L))

            # =========== PASS 1: MLP -> dy scratch ===========
            for b in range(BPC):
                for s in range(NSTRIP):
                    x3 = x3p.tile([48, 4608], F32R, tag="x3")
                    if s == 0:
                        nc.vector.memset(x3[0:48, 0:257].bitcast(F32), 0.0)
                    if s == NSTRIP - 1:
                        nc.vector.memset(x3[0:48, 4351:4608].bitcast(F32), 0.0)
                    for g in range(3):
                        S = (SROWS * s - 1) * W + (g - 1)
                        lo = max(S, 0)
                        hi = min(S + 4608, L)
                        d0, d1 = lo - S, hi - S
                        nc.sync.dma_start(
                            x3[16 * g:16 * g + 16, d0:d1],
                            _surg(x_d.bitcast(F32R), [[L, 16], [1, hi - lo]],
                                  b * CH * L + lo))
                    # wrap-column zeroing (cols that crossed a row boundary)
                    nc.sync.dma_start(
                        _surg(x3[:].bitcast(F32), [[4608, 16], [W, 18]], 0),
                        zeros_in([16, 18]))
                    nc.sync.dma_start(
                        _surg(x3[:].bitcast(F32), [[4608, 16], [W, 18]],
                              32 * 4608 + 255),
                        zeros_in([16, 18]))

                    for pair in range(TPS // 2):
                        ps1 = ps1p.tile([128, 1024], F32, tag="ps1")
                        for half in range(2):
                            j = 2 * pair + half
                            for ky in range(3):
                                nc.tensor.matmul(
                                    ps1[:, half * 512:half * 512 + 512],
                                    wtap_sb[:, ky * HID:(ky + 1) * HID],
                                    x3[0:48, (2 * j + ky) * W:(2 * j + ky) * W + 512],
                                    start=(ky == 0), stop=(ky == 2))
                        h1 = h1p.tile([128, 1024], F32R, tag="h1")
                        nc.scalar.activation(h1[:], ps1[:], AF.Lrelu,
                                             bias=b1_sb[:], scale=1.0)
                        ps2 = ps2p.tile([128, 1024], F32, tag="ps2")
                        for half in range(2):
                            nc.tensor.matmul(
                                ps2[:, half * 512:half * 512 + 512],
                                w2t_sb[:],
                                h1[:, half * 512:half * 512 + 512],
                                start=True, stop=True)
                        h2 = h2p.tile([128, 1024], F32R, tag="h2")
                        nc.scalar.activation(h2[:], ps2[:], AF.Lrelu,
                                             bias=b2_sb[:], scale=1.0)
                        ps3 = ps3p.tile([128, 1024], F32, tag="ps3")
                        for half in range(2):
                            nc.tensor.matmul(
                                ps3[:, half * 512:half * 512 + 512],
                                w3t_sb[:],
                                h2[:, half * 512:half * 512 + 512],
                                start=True, stop=True)
                        h3 = h3p.tile([128, 1024], BF16, tag="h3")
                        nc.scalar.activation(h3[:], ps3[:], AF.Lrelu,
                                             bias=b3_sb[:], scale=1.0)
                        for half in range(2):
                            j = 2 * pair + half
                            if j % 4 == 0:
                                dy4 = dy4p.tile([128, 512], F32, tag="dy4")
                            g4 = j % 4
                            nc.tensor.matmul(
                                dy4[32 * g4:32 * g4 + 16, :],
                                w4t_sb[:],
                                h3[:, half * 512:half * 512 + 512],
                                start=True, stop=True,
                                tile_position=(0, 32 * g4))
                            if j % 4 == 3:
                                dy_sb = dysbp.tile([128, 512], F32, tag="dysb")
                                nc.vector.tensor_copy(dy_sb[:], dy4[:])
                                P = (b * 32 + s * 2 + j // 4)
                                nc.sync.dma_start(
                                    _surg(dy_s[:],
                                          [[32 * 512, 4], [512, 32], [1, 512]],
                                          P * 4 * 32 * 512),
                                    dy_sb[:])

            # =========== PASS 2 ===========
            for b in range(BPC):
                xn_tiles = []
                for q in range(NQ):
                    px0 = q * (L // NQ)
                    x_pack = p2big.tile([128, F2], F32, tag="p2")
                    nc.sync.dma_start(
                        x_pack[:],
                        _surg(x_d, [[L, 16], [F2, 8], [1, F2]],
                              b * CH * L + px0))
                    dy_pack = p2big.tile([128, F2], F32, tag="p2")
                    nc.sync.dma_start(
                        dy_pack[:],
                        _surg(dy_s[:],
                              [[512, 16], [4 * 16384, 8], [16384, 4], [1, 512]],
                              (b * 32 + q * 8) * 4 * 32 * 512))
                    mask_rep = p2big.tile([128, F2], F32, tag="p2")
                    nc.sync.dma_start(
                        mask_rep[:],
                        _surg(mask_d, [[0, 16], [F2, 8], [1, F2]],
                              b * L + px0))
                    # dy *= mask ; xn = x + dy   (both in place)
                    nc.vector.tensor_tensor(dy_pack[:], dy_pack[:],
                                            mask_rep[:], OP.mult)
                    nc.vector.tensor_tensor(x_pack[:], x_pack[:],
                                            dy_pack[:], OP.add)
                    xn = x_pack  # renamed: x_pack now holds x_new
                    xn_tiles.append(xn)
                    # write alpha channels of xn into the guarded pad
                    for c in range(2):
                        nc.sync.dma_start(
                            _surg(xn_apad[:],
                                  [[8 * PITCH, 8], [PITCH, 8], [1, 256]],
                                  (b * 2 + c) * PADN + 1
                                  + (64 * q + 1) * PITCH + 1),
                            _surg(xn[:], [[F2, 8], [256, 8], [1, 256]],
                                  c * 8 * F2))

                # ---- alive masks (full batch) ----
                # 128 chunks of 2 rows; channel sections side by side in the
                # free dim (sections of 1035, mh/mv indices offset by 1035)
                res01 = []
                for pad in (x_apad, xn_apad):
                    alpha = alphap.tile([128, 2070], F32, tag="alpha")
                    for c in range(2):
                        nc.sync.dma_start(
                            alpha[:, 1035 * c:1035 * c + 1035],
                            _surg(pad[:], [[2 * PITCH, 128], [1, 1035]],
                                  (b * 2 + c) * PADN))
                    mh = mhp.tile([128, 2068], F32, tag="mh")
                    nc.vector.tensor_tensor(mh[:], alpha[:, 0:2068],
                                            alpha[:, 1:2069], OP.max)
                    nc.vector.tensor_tensor(mh[:], mh[:],
                                            alpha[:, 2:2070], OP.max)
                    mv = mvp.tile([128, 1032], F32, tag="mv")
                    for c in range(2):
                        o_mh, o_mv = 1035 * c, 516 * c
                        nc.vector.tensor_tensor(
                            mv[:, o_mv:o_mv + 516],
                            mh[:, o_mh + 1:o_mh + 517],
                            mh[:, o_mh + 259:o_mh + 775], OP.max)
                        nc.vector.tensor_tensor(
                            mv[:, o_mv:o_mv + 516],
                            mv[:, o_mv:o_mv + 516],
                            mh[:, o_mh + 517:o_mh + 1033], OP.max)
                    # abs in place, then s = |a0|+|a1|, then threshold
                    nc.scalar.activation(mv[:], mv[:], AF.Abs)
                    s01 = s01p.tile([128, 516], F32, tag="s01")
                    nc.vector.tensor_tensor(s01[:], mv[:, 0:516],
                                            mv[:, 516:1032], OP.add)
                    nc.vector.tensor_scalar(s01[:], s01[:], 0.01, None,
                                            OP.is_gt)
                    res01.append(s01)
                alive01 = res01[0]
                nc.vector.tensor_tensor(alive01[:], res01[0][:],
                                        res01[1][:], OP.mult)
                nc.sync.dma_start(
                    _surg(alive_lin[:], [[512, 128], [256, 2], [1, 256]],
                          b * L),
                    _surg(alive01[:], [[516, 128], [PITCH, 2], [1, 256]], 0))

                # ---- final multiply + output ----
                for q in range(NQ):
                    px0 = q * (L // NQ)
                    alive_rep = p2big.tile([128, F2], F32, tag="p2")
                    nc.sync.dma_start(
                        alive_rep[:],
                        _surg(alive_lin[:], [[0, 16], [F2, 8], [1, F2]],
                              b * L + px0))
                    xn = xn_tiles[q]
                    nc.vector.tensor_tensor(xn[:], xn[:], alive_rep[:],
                                            OP.mult)
                    nc.sync.dma_start(
                        _surg(out_d, [[L, 16], [F2, 8], [1, F2]],
                              b * CH * L + px0),
                        xn[:])

    nc.compile()
    return nc


_CACHE = {}
RUN_KWARGS = {}       # test harness may set {"trace": True}
LAST_RESULTS = None


def _get_nc():
    if "nc" not in _CACHE:
        _CACHE["nc"] = _build()
    return _CACHE["nc"]


def _fold_wtap(w1):
    """wtap[ky][16*kx + c, o] = sum_j w1[o, 4c+j] * f_j[ky, kx]"""
    ident = np.zeros((3, 3), np.float32); ident[1, 1] = 1.0
    sx = np.array([[-1, 0, 1], [-2, 0, 2], [-1, 0, 1]], np.float32)
    sy = sx.T.copy()
    lap = np.array([[1, 1, 1], [1, -8, 1], [1, 1, 1]], np.float32)
    filts = np.stack([ident, sx, sy, lap])            # [4, 3, 3]
    w1r = w1.reshape(HID, CH, 4)                      # [o, c, j]
    # wtap[ky, kx, c, o] = sum_j w1r[o, c, j] * filts[j, ky, kx]
    wt = np.einsum("ocj,jyx->yxco", w1r, filts)       # [ky, kx, c, o]
    return np.ascontiguousarray(wt.reshape(3, 48, HID).astype(np.float32))


def _one_step(x, w1, b1, w2, b2, w3, b3, w4, update_mask):
    nc = _get_nc()
    wtap = _fold_wtap(np.asarray(w1, np.float32))
    w2t = np.ascontiguousarray(np.asarray(w2, np.float32).T)
    w3t = np.ascontiguousarray(np.asarray(w3, np.float32).T)
    import ml_dtypes
    w4t = np.ascontiguousarray(np.asarray(w4, np.float32).T.astype(ml_dtypes.bfloat16))
    b1c = np.ascontiguousarray(np.asarray(b1, np.float32).reshape(HID, 1))
    b2c = np.ascontiguousarray(np.asarray(b2, np.float32).reshape(HID, 1))
    b3c = np.ascontiguousarray(np.asarray(b3, np.float32).reshape(HID, 1))
    in_maps = []
    for i in range(NCORES):
        xi = np.ascontiguousarray(
            x[i * BPC:(i + 1) * BPC], np.float32).reshape(-1)
        mi = np.ascontiguousarray(
            update_mask[i * BPC:(i + 1) * BPC], np.float32).reshape(-1)
        in_maps.append({
            "x": xi, "mask": mi, "wtap": wtap, "w2t": w2t, "w3t": w3t,
            "w4t": w4t, "b1": b1c, "b2": b2c, "b3": b3c,
        })
    res = run_bass_kernel_spmd(nc, in_maps, core_ids=list(range(NCORES)),
                               **RUN_KWARGS)
    globals()["LAST_RESULTS"] = res
    out = np.empty((B, CH, H, W), np.float32)
    for i in range(NCORES):
        out[i * BPC:(i + 1) * BPC] = res.results[i]["out"].reshape(
            BPC, CH, H, W)
    return out


def kernel(x, w1, b1, w2, b2, w3, b3, w4, update_mask, steps):
    x = np.asarray(x, np.float32)
    n = int(np.asarray(steps))
    cur = x
    for _ in range(n):
        cur = _one_step(cur, w1, b1, w2, b2, w3, b3, w4, update_mask)
    if n == 0:
        cur = x.copy()
    return cur

